# revision 12
# baseline (speedup 1.0000x reference)
"""MaxPool3d (kernel=3, stride=2, padding=1) on Trainium2, 8 NeuronCores.

Input  x: (2, 32, 128, 128, 128) f32  ->  Output: (2, 32, 64, 64, 64) f32.

Sharding: the 64 (b, c) slices are data-parallel; each of the 8 cores gets 8
slices, processed as 4 slice-pairs (a pair packs 2 slices into the 128 SBUF
partitions).

Per-core algorithm (separable max pooling W -> H -> D), v7:
  - Load both d-parity slabs of an h-chunk in ONE DMA (4 MiB steady;
    1/3 MiB ramp chunks for the very first rows so compute starts early):
    even-d rows land at partition 64*s + d/2 ("E" slab), odd-d at the same
    partition ("O" slab), making the final D-axis pooling partition-aligned.
  - W pool (DVE, f32 in -> fp16 out, per chunk): F = max(x[..., 0::2],
    x[..., 1::2]); F[..., 1:] = max(F[..., 1:], x[..., 1:126:2]).  fp16
    from here on: the only rounding step (rel err <= 2^-11), and every
    later tensor_tensor runs in the DVE's 2x_1P packed mode (half cycles).
  - H pool (DVE, fp16 2x) per 32-output-row half-pair, reading the pair's
    full-H F tiles (no chunk-boundary fixups): G = max(F[0::2], F[1::2]);
    G[1:] = max(G[1:], F[2*oh0-1 : 2*oh1-2 : 2]).  Slab E writes straight
    into the pair's output tile Et; slab O into Go.
  - D pool per half: Et = max(Et, Go) (fp16 2x); the 2*od-1 term is 4 PE
    matmuls with a 0/1 partition-shift matrix (exact passthrough) into 4
    PSUM banks, one ACT copy to fp16, one more fp16 DVE max.  The final
    max is emitted AFTER the next two W chunks so the PE/ACT latency hides
    behind W work instead of stalling the DVE.
  - Store per half: Et rows fp16 -> y f32, cast during an SWDGE DMA
    (Q7-generated descriptors; compute engines never touch the cast).

Engine budget/core: DVE ~195 us busy, ACT ~18 us, PE ~20 us; DMA 64 MiB in
+ 8 MiB out of HBM at ~370 GB/s (4 MiB transfers) ~ 195 us = the roofline.
"""

import os
import sys

sys.path.insert(0, "/opt/trn_rl_repo")

import numpy as np

# Shapes (hardcoded per problem spec)
B, C, D, H, W = 2, 32, 128, 128, 128
OD, OH, OW = 64, 64, 64
N_CORES = 8
SLICES_PER_CORE = (B * C) // N_CORES  # 8
PAIRS = SLICES_PER_CORE // 2  # 4
HC = 32  # max h rows per load chunk
# ramp-friendly schedule: small first chunks (pair 0 only) so DVE starts
# early.  Half-pair boundaries (64 rows) must align with chunk boundaries.
CHUNK_SIZES_RAMP = [8, 8, 16, 32, 32, 32]
CHUNK_SIZES_STEADY = [32, 32, 32, 32]
assert sum(CHUNK_SIZES_RAMP) == H and max(CHUNK_SIZES_RAMP) == HC
assert sum(CHUNK_SIZES_STEADY) == H

_cache = {}


def _shift_matrix() -> np.ndarray:
    """lhsT for the PE partition shift: out[m] = Go[m-1] within each 64-row
    slice block, with rows 0 and 64 passed through unshifted (their max
    contribution is idempotent)."""
    s = np.zeros((128, 128), dtype=np.float16)
    for m in range(128):
        k = m - 1 if m % 64 != 0 else m
        s[k, m] = 1.0
    return s


def _build():
    import concourse.mybir as mybir
    from concourse import bacc
    from concourse.tile import TileContext

    f32 = mybir.dt.float32
    f16 = mybir.dt.float16
    nc = bacc.Bacc()
    x_ext = nc.declare_dram_parameter(
        "x_shard", [SLICES_PER_CORE, D, H, W], f32, isOutput=False
    )
    smat_ext = nc.declare_dram_parameter("smat", [128, 128], f16, isOutput=False)
    y_ext = nc.declare_dram_parameter(
        "y_shard", [SLICES_PER_CORE, OD, OH, OW], f32, isOutput=True
    )

    with TileContext(nc) as tc:
        with (
            tc.tile_pool(name="cpool", bufs=1) as cpool,
            tc.tile_pool(name="xpool", bufs=3) as xpool,
            tc.tile_pool(name="fpool", bufs=1) as fpool,
            tc.tile_pool(name="gpool", bufs=2) as gpool,
            tc.tile_pool(name="spool", bufs=2) as spool,
            tc.tile_pool(name="opool", bufs=2) as opool,
            tc.tile_pool(name="ypool", bufs=2) as ypool,
            tc.tile_pool(name="ppool", bufs=2, space="PSUM") as ppool,
        ):
            smat = cpool.tile([128, 128], f16, name="smat", tag="smat")
            # SWDGE: keeps both HWDGE rings free for the first x loads
            nc.gpsimd.dma_start(out=smat[:, :], in_=smat_ext[:, :])

            dma_rr = [0]

            def load_engine():
                # alternate between the two HWDGE rings
                dma_rr[0] ^= 1
                return nc.sync if dma_rr[0] else nc.scalar

            for p in range(PAIRS):
                s0 = 2 * p
                # per-pair fp16 W-pool results, full H rows
                Fe = fpool.tile([128, H, OW], f16, name="Fe", tag="Fe")
                Fo = fpool.tile([128, H, OW], f16, name="Fo", tag="Fo")
                Ft = {0: Fe, 1: Fo}
                Et = opool.tile([128, OH, OW], f16, name="Et", tag="Et")
                xin = x_ext[s0 : s0 + 2].rearrange(
                    "s (od par) h w -> s od par h w", par=2
                )
                sizes = CHUNK_SIZES_RAMP if p == 0 else CHUNK_SIZES_STEADY
                # split chunks into the two 64-input-row halves
                halves = [[], []]
                h0 = 0
                for hc in sizes:
                    halves[0 if h0 < 64 else 1].append((h0, hc))
                    h0 += hc
                assert halves[0][-1][0] + halves[0][-1][1] == 64

                def emit_chunks(chunk_list):
                    for h0, hc in chunk_list:
                        xt = xpool.tile(
                            [128, 2, HC, W], f32, name="xt", tag="xt"
                        )
                        load_engine().dma_start(
                            out=xt[:, :, 0:hc, :],
                            in_=xin[:, :, :, h0 : h0 + hc, :],
                        )
                        for par in (0, 1):
                            F = Ft[par]
                            nc.vector.tensor_max(
                                out=F[:, h0 : h0 + hc, :],
                                in0=xt[:, par, 0:hc, 0:W:2],
                                in1=xt[:, par, 0:hc, 1:W:2],
                            )
                            nc.vector.tensor_max(
                                out=F[:, h0 : h0 + hc, 1:OW],
                                in0=F[:, h0 : h0 + hc, 1:OW],
                                in1=xt[:, par, 0:hc, 1 : W - 2 : 2],
                            )

                def emit_h_and_shift(half):
                    # output rows [oh0, oh1); input F rows [2*oh0, 2*oh1)
                    oh0, oh1 = 32 * half, 32 * (half + 1)
                    Go = gpool.tile([128, 32, OW], f16, name="Go", tag="Go")
                    for par, Gt, g0 in ((0, Et, oh0), (1, Go, 0)):
                        F = Ft[par]
                        nc.vector.tensor_max(
                            out=Gt[:, g0 : g0 + 32, :],
                            in0=F[:, 2 * oh0 : 2 * oh1 : 2, :],
                            in1=F[:, 2 * oh0 + 1 : 2 * oh1 : 2, :],
                        )
                        # odd term: F rows 2*oh-1 for oh in [max(oh0,1), oh1)
                        lo = max(oh0, 1)
                        nc.vector.tensor_max(
                            out=Gt[:, g0 + lo - oh0 : g0 + 32, :],
                            in0=Gt[:, g0 + lo - oh0 : g0 + 32, :],
                            in1=F[:, 2 * lo - 1 : 2 * oh1 - 2 : 2, :],
                        )
                    # aligned D term now; shifted term via PE+ACT (folded
                    # later, after more W work has been queued)
                    nc.vector.tensor_max(
                        out=Et[:, oh0:oh1, :],
                        in0=Et[:, oh0:oh1, :],
                        in1=Go[:, :, :],
                    )
                    Gp = ppool.tile([128, 32, OW], f32, name="Gp", tag="Gp")
                    for g0r in range(0, 32, 8):
                        nc.tensor.matmul(
                            out=Gp[:, g0r : g0r + 8, :],
                            lhsT=smat[:, :],
                            rhs=Go[:, g0r : g0r + 8, :],
                            start=True,
                            stop=True,
                        )
                    Gs = spool.tile([128, 32, OW], f16, name="Gs", tag="Gs")
                    nc.scalar.copy(out=Gs[:, :, :], in_=Gp[:, :, :])
                    return Gs

                def emit_d_fold_and_store(half, Gs):
                    oh0, oh1 = 32 * half, 32 * (half + 1)
                    nc.vector.tensor_max(
                        out=Et[:, oh0:oh1, :],
                        in0=Et[:, oh0:oh1, :],
                        in1=Gs[:, :, :],
                    )
                    # cast fp16 -> f32 on the idle ACT engine, then store
                    # via HWDGE (the SWDGE cast path runs at only ~180 GB/s
                    # and drags concurrent load throughput down)
                    Yst = ypool.tile([128, 32, OW], f32, name="Yst", tag="Yst")
                    nc.scalar.copy(out=Yst[:, :, :], in_=Et[:, oh0:oh1, :])
                    # SWDGE ring: a store waiting on D2 must not head-of-line
                    # block the HWDGE load FIFOs
                    nc.gpsimd.dma_start(
                        out=y_ext[s0 : s0 + 2, :, oh0:oh1, :],
                        in_=Yst[:, :, :],
                    )

                emit_chunks(halves[0])
                gs0 = emit_h_and_shift(0)
                emit_chunks(halves[1])
                emit_d_fold_and_store(0, gs0)
                gs1 = emit_h_and_shift(1)
                emit_d_fold_and_store(1, gs1)
    nc.compile()
    return nc


def _get_nc():
    if "nc" not in _cache:
        _cache["nc"] = _build()
    return _cache["nc"]


def run(x: np.ndarray, **spmd_kwargs):
    """Run the SPMD kernel; returns the BassKernelResults (for tracing)."""
    from concourse.bass_utils import run_bass_kernel_spmd

    nc = _get_nc()
    xs = np.ascontiguousarray(x, dtype=np.float32).reshape(B * C, D, H, W)
    smat = _shift_matrix()
    in_maps = [
        {
            "x_shard": np.ascontiguousarray(
                xs[SLICES_PER_CORE * i : SLICES_PER_CORE * (i + 1)]
            ),
            "smat": smat,
        }
        for i in range(N_CORES)
    ]
    return run_bass_kernel_spmd(nc, in_maps, list(range(N_CORES)), **spmd_kwargs)


def kernel(x: np.ndarray) -> np.ndarray:
    res = run(x)
    out = np.stack([res.results[i]["y_shard"] for i in range(N_CORES)])
    return out.reshape(B, C, OD, OH, OW)
